# revision 1
# baseline (speedup 1.0000x reference)
"""CFR network (moe_routing) Trainium2 Bass kernel.

Strategy:
  - Pure data parallel over 8 NeuronCores; MoE routing (treat/control expert
    selection) is done host-side by stable-sorting rows on t so each core
    processes a contiguous treat block then a contiguous control block and
    only ever runs ONE expert MLP per row (halves expert compute vs the
    dense reference).
  - Feature-major activations on chip: weights are the stationary matmul
    operand, activations stream as rhs [features(K) x batch(N)].  Everything
    is fp16 (matmul accumulates fp32 into PSUM).
  - ELU computed exactly via: elu(z)+1 = max(z+b+1, min(exp(z+b), 1))
    using one ScalarE Exp pass (fused bias, PSUM read), one bias-add
    extraction pass (split ACT/DVE for engine balance) and one fused DVE
    scalar_tensor_tensor combine.  Layers internally carry h' = elu+1 and
    fold the -1 into the next layer's bias (b_eff = b - W @ 1).
"""

import math
import os
from contextlib import ExitStack

import numpy as np

B = 262144
FEAT = 128
REP = 200
HYP = 200
NCORES = 8
QUANT = 1024  # batch columns per supertile
NH = 512      # matmul free-dim per instruction (one PSUM bank of fp32)

_F16 = np.float16

# Weight-layer order inside the packed weight tensor.
_LAYERS = ["r0", "r1", "r2", "t0", "t1", "t2", "c0", "c1", "c2"]


def _pack_host(inputs):
    """Transpose/cast weights, fold the +1 carry into biases, pack into two
    flat arrays (one fp16 weight pack, one fp32 bias pack)."""
    ws = {}
    bs = {}
    for name in _LAYERS:
        w = np.asarray(inputs[f"w_{name[0]}{name[1]}"], np.float32)
        b = np.asarray(inputs[f"b_{name[0]}{name[1]}"], np.float32)
        # layers whose input is h' = h + 1 need b_eff = b - W.sum(axis=1)
        if name != "r0":
            b = b - w.sum(axis=1)
        ws[name] = w.T.astype(_F16)  # [d_in, d_out] = lhsT
        bs[name] = b

    w_o = np.asarray(inputs["w_o"], np.float32)
    b_o = np.asarray(inputs["b_o"], np.float32)
    b_o_eff = float(b_o[0] - w_o.sum())
    wsT_o = w_o.T.astype(_F16)  # [200, 1]

    # ---- weight pack [128, WCOLS] fp16 ----
    cols = []
    wcols = {}
    off = 0

    def add(name, arr):
        nonlocal off
        k, m = arr.shape
        pad = np.zeros((128, m), _F16)
        pad[:k] = arr
        cols.append(pad)
        wcols[name] = (off, k, m)
        off += m

    add("r0", ws["r0"])  # [128, 200]
    for name in _LAYERS[1:]:
        add(name + "_hi", ws[name][:128])   # [128, 200]
        add(name + "_lo", ws[name][128:])   # [72, 200]
    add("o_hi", wsT_o[:128])  # [128, 1]
    add("o_lo", wsT_o[128:])  # [72, 1]
    wpack = np.concatenate(cols, axis=1)

    # ---- bias pack [128, nb] fp32: per layer per chunk, (b_eff, b_eff+1) ----
    bcols = []
    bcol_idx = {}
    for name in _LAYERS:
        b = bs[name]
        for chunk, sl in (("hi", slice(0, 128)), ("lo", slice(128, 200))):
            for var, delta in (("e", 0.0), ("c", 1.0)):
                v = np.zeros((128,), np.float32)
                seg = b[sl] + delta
                v[: len(seg)] = seg
                bcol_idx[(name, chunk, var)] = len(bcols)
                bcols.append(v)
    bpack = np.stack(bcols, axis=1).astype(np.float32)  # [128, nb]

    return wpack, wcols, bpack, bcol_idx, b_o_eff


def _build_program(R, n_treat_tiles, wcols, bcol_idx, b_o_eff, wcols_total, nb):
    """Builds the single-core Bass/Tile program (shared SPMD across cores)."""
    import concourse.bass as bass
    import concourse.tile as tile
    from concourse import bacc, mybir

    fp16 = mybir.dt.float16
    f32 = mybir.dt.float32
    Alu = mybir.AluOpType
    Act = mybir.ActivationFunctionType

    nc = bacc.Bacc("TRN2", target_bir_lowering=False, debug=False,
                   enable_asserts=False)

    x_d = nc.dram_tensor("xT", [FEAT, R], fp16, kind="ExternalInput")
    w_d = nc.dram_tensor("wpack", [128, wcols_total], fp16, kind="ExternalInput")
    b_d = nc.dram_tensor("bpack", [128, nb], f32, kind="ExternalInput")
    r_d = nc.dram_tensor("rT", [REP, R], fp16, kind="ExternalOutput")
    y_d = nc.dram_tensor("yf", [1, R], f32, kind="ExternalOutput")

    n_tiles = R // QUANT

    with tile.TileContext(nc) as tc, ExitStack() as ctx:
        wpool = ctx.enter_context(tc.tile_pool(name="w", bufs=1))
        xpool = ctx.enter_context(tc.tile_pool(name="x", bufs=4))
        hpool = ctx.enter_context(tc.tile_pool(name="h", bufs=3))
        epool = ctx.enter_context(tc.tile_pool(name="e", bufs=3))
        opool = ctx.enter_context(tc.tile_pool(name="o", bufs=3))
        pshi = ctx.enter_context(
            tc.tile_pool(name="pshi", bufs=2, space=bass.MemorySpace.PSUM))
        pslo = ctx.enter_context(
            tc.tile_pool(name="pslo", bufs=2, space=bass.MemorySpace.PSUM))

        wt = wpool.tile([128, wcols_total], fp16, tag="wt")
        nc.sync.dma_start(wt[:], w_d[:])
        bt = wpool.tile([128, nb], f32, tag="bt")
        nc.sync.dma_start(bt[:], b_d[:])

        def w_slice(name):
            off, k, m = wcols[name]
            return wt[0:k, off:off + m]

        def bias_ap(name, chunk, var, parts):
            c = bcol_idx[(name, chunk, var)]
            return bt[0:parts, c:c + 1]

        NHALVES = QUANT // NH

        for s in range(n_tiles):
            expert = "t" if s < n_treat_tiles else "c"
            col0 = s * QUANT

            xt = xpool.tile([FEAT, QUANT], fp16, tag="x")
            nc.sync.dma_start(xt[:], x_d[:, col0:col0 + QUANT])

            # cur = list of (tile, nparts) activation chunks (rhs for next MM)
            cur = [(xt, FEAT)]
            layer_names = ["r0", "r1", "r2",
                           expert + "0", expert + "1", expert + "2"]
            for li, lname in enumerate(layer_names):
                if lname == "r0":
                    wchunks = [("r0", None)]
                else:
                    wchunks = [(lname + "_hi", None), (lname + "_lo", None)]

                z_hi = pshi.tile([128, QUANT], f32, tag="zhi")
                z_lo = pslo.tile([72, QUANT], f32, tag="zlo")

                # matmuls: out chunk m, K-chunks accumulate
                for mi, (ztile, mc, msl) in enumerate(
                        ((z_hi, 128, slice(0, 128)),
                         (z_lo, 72, slice(128, 200)))):
                    for nh in range(NHALVES):
                        nsl = slice(nh * NH, (nh + 1) * NH)
                        nk = len(cur)
                        for ki, (rt, kc) in enumerate(cur):
                            wname = wchunks[ki][0]
                            off, k, m = wcols[wname]
                            lhsT = wt[0:k, off + msl.start: off + msl.start + mc]
                            nc.tensor.matmul(
                                ztile[:, nsl], lhsT, rt[:, nsl],
                                start=(ki == 0), stop=(ki == nk - 1))

                # elementwise: h' = max(z+b+1, min(exp(z+b), 1))
                new = []
                for mi, (ztile, mc, chunk) in enumerate(
                        ((z_hi, 128, "hi"), (z_lo, 72, "lo"))):
                    et = epool.tile([mc, QUANT], fp16, tag=f"e{mi}")
                    nc.scalar.activation(
                        et[:], ztile[:], Act.Exp,
                        bias=bias_ap(lname, chunk, "e", mc))
                    ct = epool.tile([mc, QUANT], fp16, tag=f"c{mi}")
                    # balance: route some extractions to ACT, rest to DVE
                    if chunk == "lo" and li >= 1:
                        nc.scalar.activation(
                            ct[:], ztile[:], Act.Identity,
                            bias=bias_ap(lname, chunk, "c", mc))
                    else:
                        nc.vector.tensor_scalar(
                            ct[:], ztile[:], bias_ap(lname, chunk, "c", mc),
                            None, Alu.add)
                    ht = hpool.tile([mc, QUANT], fp16, tag=f"h{mi}")
                    nc.vector.scalar_tensor_tensor(
                        ht[:], et[:], 1.0, ct[:], Alu.min, Alu.max)
                    new.append((ht, mc))
                cur = new

                if lname == "r2":
                    # store r = h' - 1 (fp16)
                    for mi, (ht, mc) in enumerate(cur):
                        ro = opool.tile([mc, QUANT], fp16, tag=f"r{mi}")
                        nc.vector.tensor_scalar(ro[:], ht[:], -1.0, None,
                                                Alu.add)
                        r0_ = mi * 128
                        nc.sync.dma_start(
                            r_d[r0_:r0_ + mc, col0:col0 + QUANT], ro[:])

            # final projection yf = w_o . h + b_o_eff
            zy = pshi.tile([1, QUANT], f32, tag="zhi")
            for nh in range(NHALVES):
                nsl = slice(nh * NH, (nh + 1) * NH)
                for ki, (rt, kc) in enumerate(cur):
                    wname = "o_hi" if ki == 0 else "o_lo"
                    off, k, m = wcols[wname]
                    nc.tensor.matmul(
                        zy[:, nsl], wt[0:k, off:off + 1], rt[:, nsl],
                        start=(ki == 0), stop=(ki == len(cur) - 1))
            yo = opool.tile([1, QUANT], f32, tag="y")
            nc.vector.tensor_scalar(yo[:], zy[:], b_o_eff, None, Alu.add)
            nc.sync.dma_start(y_d[:, col0:col0 + QUANT], yo[:])

    nc.compile()
    return nc


def _route(t):
    """Host-side MoE routing: stable sort by t, even split over cores,
    pad each core's treat/control block up to a QUANT multiple."""
    order = np.argsort(t, kind="stable")
    n_t = int((t == 0).sum())
    n_c = t.shape[0] - n_t

    def cap(n):
        if n == 0:
            return 0
        return int(math.ceil(n / (NCORES * QUANT))) * QUANT

    NT, NC = cap(n_t), cap(n_c)
    tchunks, cchunks = [], []
    for c in range(NCORES):
        lo = min(c * NT, n_t)
        hi = min((c + 1) * NT, n_t)
        tchunks.append(order[lo:hi])
        lo = min(c * NC, n_c)
        hi = min((c + 1) * NC, n_c)
        cchunks.append(order[n_t + lo:n_t + hi])
    return NT, NC, tchunks, cchunks


def kernel(**inputs):
    x = np.asarray(inputs["x"], np.float32)
    t = np.asarray(inputs["t"]).astype(np.int64)
    assert x.shape == (B, FEAT)

    wpack, wcols, bpack, bcol_idx, b_o_eff = _pack_host(inputs)
    NT, NC, tchunks, cchunks = _route(t)
    R = NT + NC

    x16 = x.astype(_F16)
    in_maps = []
    for c in range(NCORES):
        xc = np.zeros((FEAT, R), _F16)
        tc_, cc_ = tchunks[c], cchunks[c]
        if len(tc_):
            xc[:, :len(tc_)] = x16[tc_].T
        if len(cc_):
            xc[:, NT:NT + len(cc_)] = x16[cc_].T
        in_maps.append({"xT": np.ascontiguousarray(xc),
                        "wpack": wpack, "bpack": bpack})

    nc = _build_program(R, NT // QUANT, wcols, bcol_idx, b_o_eff,
                        wpack.shape[1], bpack.shape[1])

    from concourse import bass_utils
    res = bass_utils.run_bass_kernel_spmd(
        nc, in_maps, list(range(NCORES))).results

    r_full = np.empty((B, REP), np.float32)
    yf_full = np.empty((B, 1), np.float32)
    for c in range(NCORES):
        rT = res[c]["rT"]
        yf = res[c]["yf"]
        tc_, cc_ = tchunks[c], cchunks[c]
        if len(tc_):
            r_full[tc_] = rT[:, :len(tc_)].T.astype(np.float32)
            yf_full[tc_, 0] = yf[0, :len(tc_)]
        if len(cc_):
            r_full[cc_] = rT[:, NT:NT + len(cc_)].T.astype(np.float32)
            yf_full[cc_, 0] = yf[0, NT:NT + len(cc_)]
    return (r_full, yf_full)


if __name__ == "__main__":
    import reference

    inputs = {k: np.asarray(v) for k, v in reference.setup_inputs().items()}
    out = kernel(**inputs)
    print("r", out[0].shape, out[0].dtype, "yf", out[1].shape, out[1].dtype)


# revision 5
# speedup vs baseline: 72.6074x; 72.6074x over previous
"""CFR network (moe_routing) Trainium2 Bass kernel.

Strategy:
  - Pure data parallel over 8 NeuronCores; MoE routing (treat/control expert
    selection) is done host-side by stable-sorting rows on t so each core
    processes a contiguous treat block then a contiguous control block and
    only ever runs ONE expert MLP per row (halves expert compute vs the
    dense reference).
  - Feature-major activations on chip: weights are the stationary matmul
    operand, activations stream as rhs [features(K) x batch(N)].  Everything
    is fp16 (matmul accumulates fp32 into PSUM).
  - ELU computed exactly via: elu(z)+1 = max(z+b+1, min(exp(z+b), 1))
    using one ScalarE Exp pass (fused bias, PSUM read), one bias-add
    extraction pass (split ACT/DVE for engine balance) and one fused DVE
    scalar_tensor_tensor combine.  Layers internally carry h' = elu+1 and
    fold the -1 into the next layer's bias (b_eff = b - W @ 1).
"""

import math
import os
from contextlib import ExitStack

import numpy as np

B = 262144
FEAT = 128
REP = 200
HYP = 200
NCORES = 8
QUANT = 1024  # batch columns per supertile
NH = 512      # matmul free-dim per instruction (one PSUM bank of fp32)

_F16 = np.float16

# Weight-layer order inside the packed weight tensor.
_LAYERS = ["r0", "r1", "r2", "t0", "t1", "t2", "c0", "c1", "c2"]


def _pack_host(inputs):
    """Transpose/cast weights, fold the +1 carry into biases, pack into two
    flat arrays (one fp16 weight pack, one fp32 bias pack)."""
    ws = {}
    bs = {}
    for name in _LAYERS:
        w = np.asarray(inputs[f"w_{name[0]}{name[1]}"], np.float32)
        b = np.asarray(inputs[f"b_{name[0]}{name[1]}"], np.float32)
        # layers whose input is h' = h + 1 need b_eff = b - W.sum(axis=1)
        if name != "r0":
            b = b - w.sum(axis=1)
        ws[name] = w.T.astype(_F16)  # [d_in, d_out] = lhsT
        bs[name] = b

    w_o = np.asarray(inputs["w_o"], np.float32)
    b_o = np.asarray(inputs["b_o"], np.float32)
    b_o_eff = float(b_o[0] - w_o.sum())
    wsT_o = w_o.T.astype(_F16)  # [200, 1]

    # ---- weight pack [128, WCOLS] fp16 ----
    cols = []
    wcols = {}
    off = 0

    def add(name, arr):
        nonlocal off
        k, m = arr.shape
        pad = np.zeros((128, m), _F16)
        pad[:k] = arr
        cols.append(pad)
        wcols[name] = (off, k, m)
        off += m

    add("r0", ws["r0"])  # [128, 200]
    for name in _LAYERS[1:]:
        add(name + "_hi", ws[name][:128])   # [128, 200]
        add(name + "_lo", ws[name][128:])   # [72, 200]
    add("o_hi", wsT_o[:128])  # [128, 1]
    add("o_lo", wsT_o[128:])  # [72, 1]
    wpack = np.concatenate(cols, axis=1)

    # ---- bias pack [128, nb] fp32: per layer per chunk, (b_eff, b_eff+1) ----
    bcols = []
    bcol_idx = {}
    for name in _LAYERS:
        b = bs[name]
        for chunk, sl in (("hi", slice(0, 128)), ("lo", slice(128, 200))):
            for var, delta in (("e", 0.0), ("c", 1.0)):
                v = np.zeros((128,), np.float32)
                seg = b[sl] + delta
                v[: len(seg)] = seg
                bcol_idx[(name, chunk, var)] = len(bcols)
                bcols.append(v)
    bpack = np.stack(bcols, axis=1).astype(np.float32)  # [128, nb]

    return wpack, wcols, bpack, bcol_idx, b_o_eff


def _build_program(R, n_treat_tiles, wcols, bcol_idx, b_o_eff, wcols_total, nb,
                   reps=1):
    """Builds the single-core Bass/Tile program (shared SPMD across cores).

    reps>1 repeats the whole computation inside the NEFF (timing only)."""
    import concourse.bass as bass
    import concourse.tile as tile
    from concourse import bacc, mybir

    fp16 = mybir.dt.float16
    f32 = mybir.dt.float32
    Alu = mybir.AluOpType
    Act = mybir.ActivationFunctionType

    nc = bacc.Bacc("TRN2", target_bir_lowering=False, debug=False,
                   enable_asserts=False)

    x_d = nc.dram_tensor("xT", [FEAT, R], fp16, kind="ExternalInput")
    w_d = nc.dram_tensor("wpack", [128, wcols_total], fp16, kind="ExternalInput")
    b_d = nc.dram_tensor("bpack", [128, nb], f32, kind="ExternalInput")
    r_d = nc.dram_tensor("rT", [REP, R], fp16, kind="ExternalOutput")
    y_d = nc.dram_tensor("yf", [1, R], f32, kind="ExternalOutput")

    n_tiles = R // QUANT

    with tile.TileContext(nc) as tc, ExitStack() as ctx:
        wpool = ctx.enter_context(tc.tile_pool(name="w", bufs=1))
        xpool = ctx.enter_context(tc.tile_pool(name="x", bufs=4))
        hpool = ctx.enter_context(tc.tile_pool(name="h", bufs=3))
        epool = ctx.enter_context(tc.tile_pool(name="e", bufs=3))
        opool = ctx.enter_context(tc.tile_pool(name="o", bufs=3))
        pshi = ctx.enter_context(
            tc.tile_pool(name="pshi", bufs=2, space=bass.MemorySpace.PSUM))
        pslo = ctx.enter_context(
            tc.tile_pool(name="pslo", bufs=2, space=bass.MemorySpace.PSUM))

        wt = wpool.tile([128, wcols_total], fp16, tag="wt")
        nc.sync.dma_start(wt[:], w_d[:])
        bt = wpool.tile([128, nb], f32, tag="bt")
        nc.sync.dma_start(bt[:], b_d[:])

        def w_slice(name):
            off, k, m = wcols[name]
            return wt[0:k, off:off + m]

        def bias_ap(name, chunk, var, parts):
            c = bcol_idx[(name, chunk, var)]
            return bt[0:parts, c:c + 1]

        NHALVES = QUANT // NH

        def body():
            for s in range(n_tiles):
                _supertile(s)

        def _supertile(s):
            expert = "t" if s < n_treat_tiles else "c"
            col0 = s * QUANT

            xt = xpool.tile([FEAT, QUANT], fp16, tag="x")
            nc.sync.dma_start(xt[:], x_d[:, col0:col0 + QUANT])

            # cur = list of (tile, nparts) activation chunks (rhs for next MM)
            cur = [(xt, FEAT)]
            layer_names = ["r0", "r1", "r2",
                           expert + "0", expert + "1", expert + "2"]
            for li, lname in enumerate(layer_names):
                if lname == "r0":
                    wchunks = [("r0", None)]
                else:
                    wchunks = [(lname + "_hi", None), (lname + "_lo", None)]

                z_hi = pshi.tile([128, QUANT], f32, tag="zhi")
                z_lo = pslo.tile([72, QUANT], f32, tag="zlo")

                # matmuls: out chunk m, K-chunks accumulate
                for mi, (ztile, mc, msl) in enumerate(
                        ((z_hi, 128, slice(0, 128)),
                         (z_lo, 72, slice(128, 200)))):
                    for nh in range(NHALVES):
                        nsl = slice(nh * NH, (nh + 1) * NH)
                        nk = len(cur)
                        for ki, (rt, kc) in enumerate(cur):
                            wname = wchunks[ki][0]
                            off, k, m = wcols[wname]
                            lhsT = wt[0:k, off + msl.start: off + msl.start + mc]
                            nc.tensor.matmul(
                                ztile[:, nsl], lhsT, rt[:, nsl],
                                start=(ki == 0), stop=(ki == nk - 1))

                # elementwise: h' = max(z+b+1, min(exp(z+b), 1))
                new = []
                for mi, (ztile, mc, chunk) in enumerate(
                        ((z_hi, 128, "hi"), (z_lo, 72, "lo"))):
                    et = epool.tile([mc, QUANT], fp16, tag=f"e{mi}")
                    nc.scalar.activation(
                        et[:], ztile[:], Act.Exp,
                        bias=bias_ap(lname, chunk, "e", mc))
                    ct = epool.tile([mc, QUANT], fp16, tag=f"c{mi}")
                    # balance: route some extractions to ACT, rest to DVE
                    if chunk == "lo" and li >= 1:
                        nc.scalar.activation(
                            ct[:], ztile[:], Act.Identity,
                            bias=bias_ap(lname, chunk, "c", mc))
                    else:
                        nc.vector.tensor_scalar(
                            ct[:], ztile[:], bias_ap(lname, chunk, "c", mc),
                            None, Alu.add)
                    ht = hpool.tile([mc, QUANT], fp16, tag=f"h{mi}")
                    nc.vector.scalar_tensor_tensor(
                        ht[:], et[:], 1.0, ct[:], Alu.min, Alu.max)
                    new.append((ht, mc))
                cur = new

                if lname == "r2":
                    # store r = h' - 1 (fp16)
                    for mi, (ht, mc) in enumerate(cur):
                        ro = opool.tile([mc, QUANT], fp16, tag=f"r{mi}")
                        nc.vector.tensor_scalar(ro[:], ht[:], -1.0, None,
                                                Alu.add)
                        r0_ = mi * 128
                        nc.sync.dma_start(
                            r_d[r0_:r0_ + mc, col0:col0 + QUANT], ro[:])

            # final projection yf = w_o . h + b_o_eff
            zy = pshi.tile([1, QUANT], f32, tag="zhi")
            for nh in range(NHALVES):
                nsl = slice(nh * NH, (nh + 1) * NH)
                for ki, (rt, kc) in enumerate(cur):
                    wname = "o_hi" if ki == 0 else "o_lo"
                    off, k, m = wcols[wname]
                    nc.tensor.matmul(
                        zy[:, nsl], wt[0:k, off:off + 1], rt[:, nsl],
                        start=(ki == 0), stop=(ki == len(cur) - 1))
            yo = opool.tile([1, QUANT], f32, tag="y")
            nc.vector.tensor_scalar(yo[:], zy[:], b_o_eff, None, Alu.add)
            nc.sync.dma_start(y_d[:, col0:col0 + QUANT], yo[:])

        if reps == 1:
            body()
        else:
            with tc.For_i(0, reps, 1):
                body()

    nc.compile()
    return nc


def _route(t):
    """Host-side MoE routing: stable sort by t, even split over cores,
    pad each core's treat/control block up to a QUANT multiple."""
    order = np.argsort(t, kind="stable")
    n_t = int((t == 0).sum())
    n_c = t.shape[0] - n_t

    def cap(n):
        if n == 0:
            return 0
        return int(math.ceil(n / (NCORES * QUANT))) * QUANT

    NT, NC = cap(n_t), cap(n_c)
    tchunks, cchunks = [], []
    for c in range(NCORES):
        lo = min(c * NT, n_t)
        hi = min((c + 1) * NT, n_t)
        tchunks.append(order[lo:hi])
        lo = min(c * NC, n_c)
        hi = min((c + 1) * NC, n_c)
        cchunks.append(order[n_t + lo:n_t + hi])
    return NT, NC, tchunks, cchunks


def kernel(**inputs):
    x = np.asarray(inputs["x"], np.float32)
    t = np.asarray(inputs["t"]).astype(np.int64)
    assert x.shape == (B, FEAT)

    wpack, wcols, bpack, bcol_idx, b_o_eff = _pack_host(inputs)
    NT, NC, tchunks, cchunks = _route(t)
    R = NT + NC

    x16 = x.astype(_F16)
    in_maps = []
    for c in range(NCORES):
        xc = np.zeros((FEAT, R), _F16)
        tc_, cc_ = tchunks[c], cchunks[c]
        if len(tc_):
            xc[:, :len(tc_)] = x16[tc_].T
        if len(cc_):
            xc[:, NT:NT + len(cc_)] = x16[cc_].T
        in_maps.append({"xT": np.ascontiguousarray(xc),
                        "wpack": wpack, "bpack": bpack})

    nc = _build_program(R, NT // QUANT, wcols, bcol_idx, b_o_eff,
                        wpack.shape[1], bpack.shape[1])

    from concourse import bass_utils
    res = bass_utils.run_bass_kernel_spmd(
        nc, in_maps, list(range(NCORES))).results

    r_full = np.empty((B, REP), np.float32)
    yf_full = np.empty((B, 1), np.float32)
    for c in range(NCORES):
        rT = res[c]["rT"]
        yf = res[c]["yf"]
        tc_, cc_ = tchunks[c], cchunks[c]
        if len(tc_):
            r_full[tc_] = rT[:, :len(tc_)].T.astype(np.float32)
            yf_full[tc_, 0] = yf[0, :len(tc_)]
        if len(cc_):
            r_full[cc_] = rT[:, NT:NT + len(cc_)].T.astype(np.float32)
            yf_full[cc_, 0] = yf[0, NT:NT + len(cc_)]
    return (r_full, yf_full)


if __name__ == "__main__":
    import reference

    inputs = {k: np.asarray(v) for k, v in reference.setup_inputs().items()}
    out = kernel(**inputs)
    print("r", out[0].shape, out[0].dtype, "yf", out[1].shape, out[1].dtype)
